# Initial kernel scaffold
#
"""DiGCN (2-layer GCNConv + parallel Linear + BatchNorm1d + ReLU) on 8 trn2 NeuronCores.

Strategy (matches the problem's sharding hint):
  - Shard nodes contiguously across 8 cores (12500 nodes/core), replicate the
    small [D,D] weights, partition edges by destination-node owner.
  - Per layer: each core computes hg = h_local @ gcn_w.T, AllGathers hg so every
    core holds the full [N,D] hg; gathers its edges' source rows via indirect
    DMA; scatter-adds into its local destination nodes via one-hot matmuls on
    the TensorEngine (edges sorted by dest tile; a host-built [128e x 128d]
    one-hot*norm matrix per 128-edge chunk turns segment-sum into PSUM
    accumulation). BN stats via a tiny AllReduce.
  - h is kept transposed in SBUF ([128 feat, nodes]) so BN reduction is a
    free-dim reduce and BN+ReLU fuse into one ScalarE activation pass.

kernel(**inputs) takes FULL inputs, returns the FULL [N,D] float32 output.
"""

import math
import os
import sys

import numpy as np

for _p in ("/opt/trn_rl_repo", "/root/.axon_site/_ro/trn_rl_repo"):
    if os.path.isdir(_p) and _p not in sys.path:
        sys.path.insert(0, _p)

# ---------------------------------------------------------------- configuration
N_GLOBAL = 100000
E_GLOBAL = 500000
D = 128
DEPTH = 2
EPS = 1e-5
NCORES = 8

F32 = None  # filled after concourse import (lazy so host-side prep is importable)
LAST_RESULTS = None  # BassKernelResults of the most recent hardware run


class _Cfg:
    def __init__(self, n_global, ncores, depth=DEPTH, eps=EPS):
        assert n_global % ncores == 0
        self.n_global = n_global
        self.ncores = ncores
        self.depth = depth
        self.eps = eps
        self.np_local = n_global // ncores          # real nodes per core
        self.nt = math.ceil(self.np_local / 128)    # dest tiles per core
        self.npad = self.nt * 128                   # padded nodes per core


# ---------------------------------------------------------------- host-side prep
def _prep_graph(cfg, edge_index, edge_weight):
    """Sort/partition edges by destination owner; build gather indices and the
    per-chunk one-hot*norm matrices. Returns (K, idx_all, mt_all, norm stats)."""
    row = np.asarray(edge_index[0], dtype=np.int64)
    col = np.asarray(edge_index[1], dtype=np.int64)
    w = np.asarray(edge_weight, dtype=np.float32)
    n = cfg.n_global

    deg = np.bincount(col, weights=w.astype(np.float64), minlength=n).astype(np.float32)
    dinv = np.where(deg > 0, 1.0 / np.sqrt(np.where(deg > 0, deg, 1.0)), 0.0).astype(
        np.float32
    )
    norm = (dinv[row] * w * dinv[col]).astype(np.float32)

    npl, nt = cfg.np_local, cfg.nt
    core = col // npl
    lc = col % npl
    gtile = core * nt + lc // 128           # global dest-tile id, 0..ncores*nt
    d_in_tile = lc % 128

    order = np.argsort(gtile, kind="stable")
    gt_s = gtile[order]
    counts = np.bincount(gtile, minlength=cfg.ncores * nt)
    starts = np.concatenate([[0], np.cumsum(counts)[:-1]])
    rank = np.arange(row.shape[0], dtype=np.int64) - starts[gt_s]

    K = max(1, int(math.ceil(counts.max() / 128)))
    nchunks = nt * K

    cw = (gt_s % nt) * K + rank // 128      # chunk index within the core
    p = rank % 128                          # partition (edge slot within chunk)
    core_s = gt_s // nt

    idx_all = np.zeros((cfg.ncores, 128, nchunks), dtype=np.int32)
    mt_all = np.zeros((cfg.ncores, 128, nchunks * 128), dtype=np.float32)
    idx_all[core_s, p, cw] = row[order].astype(np.int32)
    mt_all[core_s, p, cw * 128 + d_in_tile[order]] = norm[order]
    return K, idx_all, mt_all


def _prep_inputs(cfg, K, idx_all, mt_all, x, lin_w, gcn_w, gamma, beta):
    x = np.asarray(x, dtype=np.float32)
    npl, npad = cfg.np_local, cfg.npad
    wlin = np.concatenate([lin_w[i].T for i in range(cfg.depth)], axis=1).astype(
        np.float32
    )  # [D, depth*D], column block i = lin_w[i].T  (k, o)
    wgcn = np.concatenate([gcn_w[i].T for i in range(cfg.depth)], axis=1).astype(
        np.float32
    )
    gb = np.stack(
        sum([[gamma[i], beta[i]] for i in range(cfg.depth)], []), axis=1
    ).astype(np.float32)  # [D, 2*depth]: columns g0,b0,g1,b1

    in_maps = []
    for r in range(cfg.ncores):
        xs = x[r * npl : (r + 1) * npl]
        xT = np.zeros((D, npad), dtype=np.float32)
        xT[:, :npl] = xs.T
        in_maps.append(
            {
                "xT": np.ascontiguousarray(xT),
                "wlin": np.ascontiguousarray(wlin),
                "wgcn": np.ascontiguousarray(wgcn),
                "gb": np.ascontiguousarray(gb),
                "gidx": np.ascontiguousarray(idx_all[r]),
                "mt": np.ascontiguousarray(mt_all[r]),
            }
        )
    return in_maps


# ---------------------------------------------------------------- bass program
def _build_program(cfg, K):
    from concourse import bacc, bass, mybir, tile

    f32 = mybir.dt.float32
    i32 = mybir.dt.int32
    npl, npad, nt = cfg.np_local, cfg.npad, cfg.nt
    nchunks = nt * K
    rg = [list(range(cfg.ncores))]
    inv_n = 1.0 / cfg.n_global

    # tiles-per-gather-batch: keep each G/Mt staging buffer <= ~36 chunks
    tb = max(1, 36 // K)
    nbatch = math.ceil(nt / tb)

    nc = bacc.Bacc(
        "TRN2", target_bir_lowering=False, debug=False, num_devices=cfg.ncores
    )

    xT = nc.dram_tensor("xT", [D, npad], f32, kind="ExternalInput")
    wlin_d = nc.dram_tensor("wlin", [D, cfg.depth * D], f32, kind="ExternalInput")
    wgcn_d = nc.dram_tensor("wgcn", [D, cfg.depth * D], f32, kind="ExternalInput")
    gb_d = nc.dram_tensor("gb", [D, 2 * cfg.depth], f32, kind="ExternalInput")
    gidx_d = nc.dram_tensor("gidx", [128, nchunks], i32, kind="ExternalInput")
    mt_d = nc.dram_tensor("mt", [128, nchunks * 128], f32, kind="ExternalInput")
    outT_d = nc.dram_tensor("outT", [D, npl], f32, kind="ExternalOutput")

    with tile.TileContext(nc) as tc:
        with (
            tc.tile_pool(name="big", bufs=1) as big,
            tc.tile_pool(name="gpool", bufs=2) as gpool,
            tc.tile_pool(name="mpool", bufs=2) as mpool,
            tc.tile_pool(name="cpool", bufs=4) as cpool,
            tc.tile_pool(name="small", bufs=1) as small,
            tc.tile_pool(name="stats", bufs=2) as stats_pool,
            tc.tile_pool(name="psA", bufs=4, space="PSUM") as psA,
            tc.tile_pool(name="psH", bufs=2, space="PSUM") as psH,
            tc.tile_pool(name="dram", bufs=1, space="DRAM") as dpool,
        ):
            hA = big.tile([128, npad], f32)
            hB = big.tile([128, npad], f32)
            widx = small.tile([128, nchunks], i32)
            wlin = small.tile([128, cfg.depth * D], f32)
            wgcn = small.tile([128, cfg.depth * D], f32)
            gb = small.tile([128, 2 * cfg.depth], f32)

            nc.sync.dma_start(out=hA[:, :], in_=xT[:, :])
            nc.sync.dma_start(out=widx[:, :], in_=gidx_d[:, :])
            nc.sync.dma_start(out=wlin[:, :], in_=wlin_d[:, :])
            nc.sync.dma_start(out=wgcn[:, :], in_=wgcn_d[:, :])
            nc.sync.dma_start(out=gb[:, :], in_=gb_d[:, :])

            cur, nxt = hA, hB
            for layer in range(cfg.depth):
                # ---- phase A: hg = h @ gcn_w.T (node-major tiles) -> cc_in
                cc_in = dpool.tile([npl, D], f32, name=f"cc_in_{layer}")
                cc_out = dpool.tile(
                    [cfg.n_global, D], f32, addr_space="Shared", name=f"cc_out_{layer}"
                )
                wg = wgcn[:, layer * D : (layer + 1) * D]
                for t in range(nt):
                    n0 = t * 128
                    nn = min(npl, n0 + 128) - n0
                    ps = psA.tile([128, 128], f32, name="ps_hg", tag="ps_hg")
                    nc.tensor.matmul(
                        ps[:, :],
                        lhsT=cur[:, n0 : n0 + 128],
                        rhs=wg,
                        start=True,
                        stop=True,
                    )
                    sb = cpool.tile([128, 128], f32, name="sb_hg", tag="sb_hg")
                    nc.vector.tensor_copy(sb[:, :], ps[:, :])
                    nc.sync.dma_start(out=cc_in[n0 : n0 + nn, :], in_=sb[:nn, :])

                # ---- phase B: AllGather hg
                nc.gpsimd.collective_compute(
                    "AllGather",
                    mybir.AluOpType.bypass,
                    replica_groups=rg,
                    ins=[cc_in[:, :].opt()],
                    outs=[cc_out[:, :].opt()],
                )

                # ---- phase C: hl = h @ lin_w.T  (transposed layout, into nxt)
                wl = wlin[:, layer * D : (layer + 1) * D]
                c0 = 0
                while c0 < npad:
                    cw = min(512, npad - c0)
                    ps = psH.tile([128, 512], f32, name="ps_hl", tag="ps_hl")
                    nc.tensor.matmul(
                        ps[:, :cw],
                        lhsT=wl,
                        rhs=cur[:, c0 : c0 + cw],
                        start=True,
                        stop=True,
                    )
                    nc.vector.tensor_copy(nxt[:, c0 : c0 + cw], ps[:, :cw])
                    c0 += cw

                # ---- phase D: gather + one-hot matmul scatter-add
                for b in range(nbatch):
                    t0 = b * tb
                    t1 = min(nt, t0 + tb)
                    nch = (t1 - t0) * K
                    g = gpool.tile([128, tb * K * 128], f32, name="gbuf", tag="gbuf")
                    m = mpool.tile([128, tb * K * 128], f32, name="mbuf", tag="mbuf")
                    nc.gpsimd.indirect_dma_start(
                        out=g[:, : nch * 128],
                        out_offset=None,
                        in_=cc_out[:, :],
                        in_offset=bass.IndirectOffsetOnAxis(
                            ap=widx[:, t0 * K : t0 * K + nch], axis=0
                        ),
                    )
                    nc.sync.dma_start(
                        out=m[:, : nch * 128],
                        in_=mt_d[:, t0 * K * 128 : (t0 * K + nch) * 128],
                    )
                    for t in range(t0, t1):
                        ps = psA.tile([128, 128], f32, name="ps_agg", tag="ps_hg")
                        for j in range(K):
                            cl = (t - t0) * K + j
                            nc.tensor.matmul(
                                ps[:, :],
                                lhsT=g[:, cl * 128 : (cl + 1) * 128],
                                rhs=m[:, cl * 128 : (cl + 1) * 128],
                                start=(j == 0),
                                stop=(j == K - 1),
                            )
                        nc.vector.tensor_add(
                            nxt[:, t * 128 : (t + 1) * 128],
                            nxt[:, t * 128 : (t + 1) * 128],
                            ps[:, :],
                        )

                # ---- phase E: BatchNorm stats + AllReduce
                st = stats_pool.tile([128, 2], f32, name=f"st_{layer}")
                nc.vector.reduce_sum(
                    out=st[:, 0:1], in_=nxt[:, :npl], axis=mybir.AxisListType.X
                )
                nc.scalar.activation(
                    out=cur[:, :npl],
                    in_=nxt[:, :npl],
                    func=mybir.ActivationFunctionType.Square,
                    accum_out=st[:, 1:2],
                )
                bn_in = dpool.tile([128, 2], f32, name=f"bn_in_{layer}")
                bn_out = dpool.tile(
                    [128, 2], f32, addr_space="Shared", name=f"bn_out_{layer}"
                )
                nc.sync.dma_start(out=bn_in[:, :], in_=st[:, :])
                nc.gpsimd.collective_compute(
                    "AllReduce",
                    mybir.AluOpType.add,
                    replica_groups=rg,
                    ins=[bn_in[:, :].opt()],
                    outs=[bn_out[:, :].opt()],
                )
                gst = stats_pool.tile([128, 2], f32, name=f"gst_{layer}")
                nc.sync.dma_start(out=gst[:, :], in_=bn_out[:, :])

                # scale = gamma * rsqrt(var+eps); bias = beta - mean*scale
                mu = stats_pool.tile([128, 1], f32, name=f"mu_{layer}")
                vr = stats_pool.tile([128, 1], f32, name=f"vr_{layer}")
                sc = stats_pool.tile([128, 1], f32, name=f"sc_{layer}")
                bi = stats_pool.tile([128, 1], f32, name=f"bi_{layer}")
                nc.scalar.mul(mu[:, :], gst[:, 0:1], inv_n)  # mean
                # vr = E[x^2] - mu^2 + eps
                nc.scalar.activation(
                    out=vr[:, :],
                    in_=mu[:, :],
                    func=mybir.ActivationFunctionType.Square,
                )
                nc.vector.tensor_scalar(
                    out=vr[:, :],
                    in0=gst[:, 1:2],
                    scalar1=inv_n,
                    scalar2=None,
                    op0=mybir.AluOpType.mult,
                )
                # vr now = E[x^2]; subtract mu^2, add eps, sqrt, reciprocal
                mu2 = stats_pool.tile([128, 1], f32, name=f"mu2_{layer}")
                nc.scalar.activation(
                    out=mu2[:, :],
                    in_=mu[:, :],
                    func=mybir.ActivationFunctionType.Square,
                )
                nc.vector.tensor_sub(vr[:, :], vr[:, :], mu2[:, :])
                nc.scalar.activation(
                    out=vr[:, :],
                    in_=vr[:, :],
                    func=mybir.ActivationFunctionType.Sqrt,
                    bias=float(cfg.eps),
                )
                nc.vector.reciprocal(vr[:, :], vr[:, :])  # rstd
                nc.vector.tensor_mul(sc[:, :], vr[:, :], gb[:, 2 * layer : 2 * layer + 1])
                nc.vector.tensor_mul(bi[:, :], mu[:, :], sc[:, :])
                nc.vector.tensor_sub(bi[:, :], gb[:, 2 * layer + 1 : 2 * layer + 2], bi[:, :])

                # ---- phase F: apply BN (+ReLU except last layer), into cur
                func = (
                    mybir.ActivationFunctionType.Relu
                    if layer != cfg.depth - 1
                    else mybir.ActivationFunctionType.Identity
                )
                nc.scalar.activation(
                    out=cur[:, :],
                    in_=nxt[:, :],
                    func=func,
                    bias=bi[:, :],
                    scale=sc[:, :],
                )
                # cur now holds the layer output (transposed); nxt is free
                if layer == cfg.depth - 1:
                    nc.sync.dma_start(out=outT_d[:, :], in_=cur[:, :npl])

    nc.compile()
    return nc


# ---------------------------------------------------------------- entry points
def _run_hw(cfg, nc, in_maps):
    global LAST_RESULTS
    from concourse.bass_utils import run_bass_kernel_spmd

    trace = bool(int(os.environ.get("KERNEL_TRACE", "0")))
    res = run_bass_kernel_spmd(
        nc,
        in_maps,
        core_ids=list(range(cfg.ncores)),
        trace=trace,
    )
    LAST_RESULTS = res
    return res.results


def _assemble(cfg, results):
    out = np.empty((cfg.n_global, D), dtype=np.float32)
    npl = cfg.np_local
    for r in range(cfg.ncores):
        out[r * npl : (r + 1) * npl] = results[r]["outT"].T
    return out


def kernel(x, edge_index, edge_weight, lin_w, gcn_w, gamma, beta):
    cfg = _Cfg(N_GLOBAL, NCORES)
    x = np.asarray(x)
    assert x.shape == (cfg.n_global, D)
    K, idx_all, mt_all = _prep_graph(cfg, np.asarray(edge_index), np.asarray(edge_weight))
    in_maps = _prep_inputs(
        cfg, K, idx_all, mt_all, x, np.asarray(lin_w), np.asarray(gcn_w),
        np.asarray(gamma), np.asarray(beta),
    )
    nc = _build_program(cfg, K)
    results = _run_hw(cfg, nc, in_maps)
    return _assemble(cfg, results)


# revision 10
# speedup vs baseline: 4.3318x; 4.3318x over previous
"""DiGCN (2-layer GCNConv + parallel Linear + BatchNorm1d + ReLU) on 8 trn2 NeuronCores.

Strategy (matches the problem's sharding hint):
  - Shard nodes contiguously across 8 cores (12500 nodes/core), replicate the
    small [D,D] weights, partition edges by destination-node owner.
  - Per layer: each core computes hg = h_local @ gcn_w.T, AllGathers hg so every
    core holds the full [N,D] hg; gathers its edges' source rows via indirect
    DMA; scatter-adds into its local destination nodes via one-hot matmuls on
    the TensorEngine (edges sorted by dest tile; a host-built [128e x 128d]
    one-hot*norm matrix per 128-edge chunk turns segment-sum into PSUM
    accumulation). BN stats via a tiny AllReduce.
  - h is kept transposed in SBUF ([128 feat, nodes]) so BN reduction is a
    free-dim reduce and BN+ReLU fuse into one ScalarE activation pass.

kernel(**inputs) takes FULL inputs, returns the FULL [N,D] float32 output.
"""

import math
import os
import sys

import numpy as np

for _p in ("/opt/trn_rl_repo", "/root/.axon_site/_ro/trn_rl_repo"):
    if os.path.isdir(_p) and _p not in sys.path:
        sys.path.insert(0, _p)

# ---------------------------------------------------------------- configuration
N_GLOBAL = 100000
E_GLOBAL = 500000
D = 128
DEPTH = 2
EPS = 1e-5
NCORES = 8

F32 = None  # filled after concourse import (lazy so host-side prep is importable)
LAST_RESULTS = None  # BassKernelResults of the most recent hardware run


class _Cfg:
    def __init__(self, n_global, ncores, depth=DEPTH, eps=EPS):
        assert n_global % ncores == 0
        self.n_global = n_global
        self.ncores = ncores
        self.depth = depth
        self.eps = eps
        self.np_local = n_global // ncores          # real nodes per core
        self.nt = math.ceil(self.np_local / 128)    # dest tiles per core
        self.npad = self.nt * 128                   # padded nodes per core


# ---------------------------------------------------------------- host-side prep
def _prep_graph(cfg, edge_index, edge_weight):
    """Sort/partition edges by destination owner; build gather indices and the
    per-chunk one-hot*norm matrices. Returns (K, idx_all, mt_all, norm stats)."""
    row = np.asarray(edge_index[0], dtype=np.int64)
    col = np.asarray(edge_index[1], dtype=np.int64)
    w = np.asarray(edge_weight, dtype=np.float32)
    n = cfg.n_global

    deg = np.bincount(col, weights=w.astype(np.float64), minlength=n).astype(np.float32)
    dinv = np.where(deg > 0, 1.0 / np.sqrt(np.where(deg > 0, deg, 1.0)), 0.0).astype(
        np.float32
    )
    norm = (dinv[row] * w * dinv[col]).astype(np.float32)

    npl, nt = cfg.np_local, cfg.nt
    core = col // npl
    lc = col % npl
    gtile = core * nt + lc // 128           # global dest-tile id, 0..ncores*nt
    d_in_tile = lc % 128

    order = np.argsort(gtile, kind="stable")
    gt_s = gtile[order]
    counts = np.bincount(gtile, minlength=cfg.ncores * nt)
    starts = np.concatenate([[0], np.cumsum(counts)[:-1]])
    rank = np.arange(row.shape[0], dtype=np.int64) - starts[gt_s]

    K = max(1, int(math.ceil(counts.max() / 128)))
    nchunks = nt * K

    cw = (gt_s % nt) * K + rank // 128      # chunk index within the core
    p = rank % 128                          # partition (edge slot within chunk)
    core_s = gt_s // nt

    idx_all = np.zeros((cfg.ncores, 128, nchunks), dtype=np.int32)
    mt_all = np.zeros((cfg.ncores, 128, nchunks * 128), dtype=np.float32)
    idx_all[core_s, p, cw] = row[order].astype(np.int32)
    mt_all[core_s, p, cw * 128 + d_in_tile[order]] = norm[order]
    return K, idx_all, mt_all


def _prep_inputs(cfg, K, idx_all, mt_all, x, lin_w, gcn_w, gamma, beta):
    x = np.asarray(x, dtype=np.float32)
    npl, npad = cfg.np_local, cfg.npad
    wlin = np.concatenate([lin_w[i].T for i in range(cfg.depth)], axis=1).astype(
        np.float32
    )  # [D, depth*D], column block i = lin_w[i].T  (k, o)
    wgcn = np.concatenate([gcn_w[i].T for i in range(cfg.depth)], axis=1).astype(
        np.float32
    )
    gb = np.stack(
        sum([[gamma[i], beta[i]] for i in range(cfg.depth)], []), axis=1
    ).astype(np.float32)  # [D, 2*depth]: columns g0,b0,g1,b1

    in_maps = []
    for r in range(cfg.ncores):
        xs = x[r * npl : (r + 1) * npl]
        xT = np.zeros((D, npad), dtype=np.float32)
        xT[:, :npl] = xs.T
        in_maps.append(
            {
                "xT": np.ascontiguousarray(xT),
                "wlin": np.ascontiguousarray(wlin),
                "wgcn": np.ascontiguousarray(wgcn),
                "gb": np.ascontiguousarray(gb),
                "gidx": np.ascontiguousarray(idx_all[r]),
                "mt": np.ascontiguousarray(mt_all[r]),
            }
        )
    return in_maps


# ---------------------------------------------------------------- bass program
def _build_program(cfg, K):
    from concourse import bacc, bass, mybir, tile

    f32 = mybir.dt.float32
    i32 = mybir.dt.int32
    npl, npad, nt = cfg.np_local, cfg.npad, cfg.nt
    nchunks = nt * K
    rg = [list(range(cfg.ncores))]
    inv_n = 1.0 / cfg.n_global

    # tiles-per-gather-batch: keep each G/Mt staging buffer <= ~36 chunks
    tb = max(1, 36 // K)
    nbatch = math.ceil(nt / tb)

    nc = bacc.Bacc(
        "TRN2", target_bir_lowering=False, debug=False, num_devices=cfg.ncores
    )

    xT = nc.dram_tensor("xT", [D, npad], f32, kind="ExternalInput")
    wlin_d = nc.dram_tensor("wlin", [D, cfg.depth * D], f32, kind="ExternalInput")
    wgcn_d = nc.dram_tensor("wgcn", [D, cfg.depth * D], f32, kind="ExternalInput")
    gb_d = nc.dram_tensor("gb", [D, 2 * cfg.depth], f32, kind="ExternalInput")
    gidx_d = nc.dram_tensor("gidx", [128, nchunks], i32, kind="ExternalInput")
    mt_d = nc.dram_tensor("mt", [128, nchunks * 128], f32, kind="ExternalInput")
    outT_d = nc.dram_tensor("outT", [D, npl], f32, kind="ExternalOutput")
    debug = bool(int(os.environ.get("KERNEL_DEBUG", "0")))
    skips = set(os.environ.get("KERNEL_SKIP", "").split(","))
    if debug:
        dbg_d = nc.dram_tensor("dbg", [128, 1024], f32, kind="ExternalOutput")

    with tile.TileContext(nc) as tc:
        with (
            tc.tile_pool(name="big", bufs=1) as big,
            tc.tile_pool(name="gpool", bufs=2) as gpool,
            tc.tile_pool(name="mpool", bufs=2) as mpool,
            tc.tile_pool(name="cpool", bufs=4) as cpool,
            tc.tile_pool(name="small", bufs=1) as small,
            tc.tile_pool(name="stats", bufs=2) as stats_pool,
            tc.tile_pool(name="psA", bufs=4, space="PSUM") as psA,
            tc.tile_pool(name="psH", bufs=2, space="PSUM") as psH,
            tc.tile_pool(name="dram", bufs=1, space="DRAM") as dpool,
        ):
            hA = big.tile([128, npad], f32)
            hB = big.tile([128, npad], f32)
            consts = small.tile([128, 2], f32)  # col0 = 0.0, col1 = eps
            nc.vector.memset(consts[:, 0:1], 0.0)
            nc.vector.memset(consts[:, 1:2], float(cfg.eps))
            dbg = None
            if debug:
                dbg = small.tile([128, 1024], f32)
                nc.vector.memset(dbg[:, :], 0.0)
            widx = small.tile([128, nchunks], i32)
            wlin = small.tile([128, cfg.depth * D], f32)
            wgcn = small.tile([128, cfg.depth * D], f32)
            gb = small.tile([128, 2 * cfg.depth], f32)

            nc.sync.dma_start(out=hA[:, :], in_=xT[:, :])
            nc.sync.dma_start(out=widx[:, :], in_=gidx_d[:, :])
            nc.sync.dma_start(out=wlin[:, :], in_=wlin_d[:, :])
            nc.sync.dma_start(out=wgcn[:, :], in_=wgcn_d[:, :])
            nc.sync.dma_start(out=gb[:, :], in_=gb_d[:, :])

            cur, nxt = hA, hB
            for layer in range(cfg.depth):
                # ---- phase A: hg = h @ gcn_w.T (node-major tiles) -> cc_in
                cc_in = dpool.tile([npl, D], f32, name=f"cc_in_{layer}")
                cc_out = dpool.tile(
                    [cfg.n_global, D], f32, addr_space="Shared", name=f"cc_out_{layer}"
                )
                wg = wgcn[:, layer * D : (layer + 1) * D]
                for t in range(nt):
                    n0 = t * 128
                    nn = min(npl, n0 + 128) - n0
                    ps = psA.tile([128, 128], f32, name="ps_hg", tag="ps_hg")
                    nc.tensor.matmul(
                        ps[:, :],
                        lhsT=cur[:, n0 : n0 + 128],
                        rhs=wg,
                        start=True,
                        stop=True,
                    )
                    sb = cpool.tile([128, 128], f32, name="sb_hg", tag="sb_hg")
                    nc.vector.tensor_copy(sb[:, :], ps[:, :])
                    nc.sync.dma_start(out=cc_in[n0 : n0 + nn, :], in_=sb[:nn, :])
                    if debug and layer == 0 and t == 0:
                        nc.vector.tensor_copy(dbg[:, 0:128], sb[:, :])

                # ---- phase B: AllGather hg
                if "ag" not in skips:
                    nc.gpsimd.collective_compute(
                        "AllGather",
                        mybir.AluOpType.bypass,
                        replica_groups=rg,
                        ins=[cc_in[:, :].opt()],
                        outs=[cc_out[:, :].opt()],
                    )

                if debug and layer == 0:
                    ccs = cpool.tile([128, 128], f32, name="ccs", tag="sb_hg")
                    nc.sync.dma_start(out=ccs[:, :], in_=cc_out[0:128, :])
                    nc.vector.tensor_copy(dbg[:, 128:256], ccs[:, :])

                # ---- phase C: hl = h @ lin_w.T  (transposed layout, into nxt)
                wl = wlin[:, layer * D : (layer + 1) * D]
                c0 = 0
                while c0 < npad:
                    cw = min(512, npad - c0)
                    ps = psH.tile([128, 512], f32, name="ps_hl", tag="ps_hl")
                    nc.tensor.matmul(
                        ps[:, :cw],
                        lhsT=wl,
                        rhs=cur[:, c0 : c0 + cw],
                        start=True,
                        stop=True,
                    )
                    nc.vector.tensor_copy(nxt[:, c0 : c0 + cw], ps[:, :cw])
                    c0 += cw

                # ---- phase D: gather + one-hot matmul scatter-add
                for b in range(nbatch):
                    t0 = b * tb
                    t1 = min(nt, t0 + tb)
                    nch = (t1 - t0) * K
                    g = gpool.tile([128, tb * K * 128], f32, name="gbuf", tag="gbuf")
                    m = mpool.tile([128, tb * K * 128], f32, name="mbuf", tag="mbuf")
                    if "gather" not in skips:
                        nc.gpsimd.indirect_dma_start(
                            out=g[:, : nch * 128],
                            out_offset=None,
                            in_=cc_out[:, :],
                            in_offset=bass.IndirectOffsetOnAxis(
                                ap=widx[:, t0 * K : t0 * K + nch], axis=0
                            ),
                        )
                    if "mt" not in skips:
                        nc.sync.dma_start(
                            out=m[:, : nch * 128],
                            in_=mt_d[:, t0 * K * 128 : (t0 * K + nch) * 128],
                        )
                    if debug and layer == 0 and b == 0:
                        nc.vector.tensor_copy(dbg[:, 256:384], g[:, 0:128])
                        nc.vector.tensor_copy(dbg[:, 640:768], g[:, 128:256])
                        nc.vector.tensor_copy(dbg[:, 768:896], g[:, 256:384])
                    for t in range(t0, t1):
                        if "aggmm" in skips:
                            break
                        ps = psA.tile([128, 128], f32, name="ps_agg", tag="ps_hg")
                        for j in range(K):
                            cl = (t - t0) * K + j
                            nc.tensor.matmul(
                                ps[:, :],
                                lhsT=g[:, cl * 128 : (cl + 1) * 128],
                                rhs=m[:, cl * 128 : (cl + 1) * 128],
                                start=(j == 0),
                                stop=(j == K - 1),
                            )
                        if debug and layer == 0 and t == 0:
                            nc.vector.tensor_copy(dbg[:, 896:1024], ps[:, :])
                        nc.vector.tensor_add(
                            nxt[:, t * 128 : (t + 1) * 128],
                            nxt[:, t * 128 : (t + 1) * 128],
                            ps[:, :],
                        )

                # ---- phase E: BatchNorm stats + AllReduce
                if debug and layer == 0:
                    nc.vector.tensor_copy(dbg[:, 384:512], nxt[:, 0:128])
                st = stats_pool.tile([128, 2], f32, name=f"st_{layer}")
                nc.vector.reduce_sum(
                    out=st[:, 0:1], in_=nxt[:, :npl], axis=mybir.AxisListType.X
                )
                nc.scalar.activation(
                    out=cur[:, :npl],
                    in_=nxt[:, :npl],
                    func=mybir.ActivationFunctionType.Square,
                    bias=consts[:, 0:1],
                    accum_out=st[:, 1:2],
                )
                bn_in = dpool.tile([128, 2], f32, name=f"bn_in_{layer}")
                bn_out = dpool.tile(
                    [128, 2], f32, addr_space="Shared", name=f"bn_out_{layer}"
                )
                nc.sync.dma_start(out=bn_in[:, :], in_=st[:, :])
                nc.gpsimd.collective_compute(
                    "AllReduce",
                    mybir.AluOpType.add,
                    replica_groups=rg,
                    ins=[bn_in[:, :].opt()],
                    outs=[bn_out[:, :].opt()],
                )
                gst = stats_pool.tile([128, 2], f32, name=f"gst_{layer}")
                nc.sync.dma_start(out=gst[:, :], in_=bn_out[:, :])

                # scale = gamma * rsqrt(var+eps); bias = beta - mean*scale
                mu = stats_pool.tile([128, 1], f32, name=f"mu_{layer}")
                vr = stats_pool.tile([128, 1], f32, name=f"vr_{layer}")
                sc = stats_pool.tile([128, 1], f32, name=f"sc_{layer}")
                bi = stats_pool.tile([128, 1], f32, name=f"bi_{layer}")
                nc.scalar.mul(mu[:, :], gst[:, 0:1], inv_n)  # mean
                # vr = E[x^2] - mu^2 + eps
                nc.vector.tensor_scalar(
                    out=vr[:, :],
                    in0=gst[:, 1:2],
                    scalar1=inv_n,
                    scalar2=None,
                    op0=mybir.AluOpType.mult,
                )
                # vr now = E[x^2]; subtract mu^2, add eps, sqrt, reciprocal
                mu2 = stats_pool.tile([128, 1], f32, name=f"mu2_{layer}")
                nc.scalar.activation(
                    out=mu2[:, :],
                    in_=mu[:, :],
                    func=mybir.ActivationFunctionType.Square,
                    bias=consts[:, 0:1],
                )
                nc.vector.tensor_sub(vr[:, :], vr[:, :], mu2[:, :])
                nc.scalar.activation(
                    out=vr[:, :],
                    in_=vr[:, :],
                    func=mybir.ActivationFunctionType.Sqrt,
                    bias=consts[:, 1:2],
                )
                nc.vector.reciprocal(vr[:, :], vr[:, :])  # rstd
                nc.vector.tensor_mul(sc[:, :], vr[:, :], gb[:, 2 * layer : 2 * layer + 1])
                nc.vector.tensor_mul(bi[:, :], mu[:, :], sc[:, :])
                nc.vector.tensor_sub(bi[:, :], gb[:, 2 * layer + 1 : 2 * layer + 2], bi[:, :])

                if debug and layer == 0:
                    nc.vector.tensor_copy(dbg[:, 512:514], st[:, :])
                    nc.vector.tensor_copy(dbg[:, 514:516], gst[:, :])
                    nc.vector.tensor_copy(dbg[:, 516:517], mu[:, :])
                    nc.vector.tensor_copy(dbg[:, 517:518], vr[:, :])
                    nc.vector.tensor_copy(dbg[:, 518:519], sc[:, :])
                    nc.vector.tensor_copy(dbg[:, 519:520], bi[:, :])

                # ---- phase F: apply BN (+ReLU except last layer), into cur
                func = (
                    mybir.ActivationFunctionType.Relu
                    if layer != cfg.depth - 1
                    else mybir.ActivationFunctionType.Identity
                )
                nc.scalar.activation(
                    out=cur[:, :],
                    in_=nxt[:, :],
                    func=func,
                    bias=bi[:, :],
                    scale=sc[:, :],
                )
                # cur now holds the layer output (transposed); nxt is free
                if layer == cfg.depth - 1:
                    nc.sync.dma_start(out=outT_d[:, :], in_=cur[:, :npl])
                    if debug:
                        nc.sync.dma_start(out=dbg_d[:, :], in_=dbg[:, :])

    nc.compile()
    return nc


# ---------------------------------------------------------------- entry points
def _make_runner(cfg, nc, in_maps):
    """Build a repeat-callable PJRT runner with device-resident inputs (no
    donation) for wall-clock timing. Returns (run_once, fetch_results)."""
    import jax
    from jax.experimental.shard_map import shard_map
    from jax.sharding import Mesh, NamedSharding, PartitionSpec

    from concourse import bass2jax, mybir

    bass2jax.install_neuronx_cc_hook()

    partition_name = nc.partition_id_tensor.name if nc.partition_id_tensor else None
    in_names, out_names, out_avals, zero_outs = [], [], [], []
    for alloc in nc.m.functions[0].allocations:
        if not isinstance(alloc, mybir.MemoryLocationSet):
            continue
        name = alloc.memorylocations[0].name
        if alloc.kind == "ExternalInput":
            if name != partition_name:
                in_names.append(name)
        elif alloc.kind == "ExternalOutput":
            out_names.append(name)
            shape = tuple(alloc.tensor_shape)
            dtype = mybir.dt.np(alloc.dtype)
            out_avals.append(jax.core.ShapedArray(shape, dtype))
            zero_outs.append(np.zeros(shape, dtype))
    n_params = len(in_names)
    all_in_names = list(in_names) + list(out_names)
    if partition_name is not None:
        all_in_names.append(partition_name)

    def _body(*args):
        operands = list(args)
        if partition_name is not None:
            operands.append(bass2jax.partition_id_tensor())
        outs = bass2jax._bass_exec_p.bind(
            *operands,
            out_avals=tuple(out_avals),
            in_names=tuple(all_in_names),
            out_names=tuple(out_names),
            lowering_input_output_aliases=(),
            sim_require_finite=True,
            sim_require_nnan=True,
            nc=nc,
        )
        return tuple(outs)

    n = cfg.ncores
    devices = jax.devices()[:n]
    mesh = Mesh(np.asarray(devices), ("core",))
    n_outs = len(out_names)
    in_specs = (PartitionSpec("core"),) * (n_params + n_outs)
    out_specs = (PartitionSpec("core"),) * n_outs
    sharded = jax.jit(
        shard_map(
            _body, mesh=mesh, in_specs=in_specs, out_specs=out_specs, check_rep=False
        ),
        keep_unused=True,
    )
    shd = NamedSharding(mesh, PartitionSpec("core"))
    concat_in = [
        jax.device_put(
            np.concatenate([np.asarray(in_maps[c][k]) for c in range(n)], axis=0), shd
        )
        for k in in_names
    ]
    concat_zeros = [
        jax.device_put(np.zeros((n * z.shape[0], *z.shape[1:]), z.dtype), shd)
        for z in zero_outs
    ]

    def run_once():
        outs = sharded(*concat_in, *concat_zeros)
        jax.block_until_ready(outs)
        return outs

    def fetch(outs):
        return [
            {
                k: np.asarray(outs[i]).reshape(n, *out_avals[i].shape)[c]
                for i, k in enumerate(out_names)
            }
            for c in range(n)
        ]

    return run_once, fetch


def _run_hw(cfg, nc, in_maps):
    global LAST_RESULTS
    from concourse.bass_utils import run_bass_kernel_spmd

    trace = bool(int(os.environ.get("KERNEL_TRACE", "0")))
    res = run_bass_kernel_spmd(
        nc,
        in_maps,
        core_ids=list(range(cfg.ncores)),
        trace=trace,
    )
    LAST_RESULTS = res
    return res.results


def _assemble(cfg, results):
    out = np.empty((cfg.n_global, D), dtype=np.float32)
    npl = cfg.np_local
    for r in range(cfg.ncores):
        out[r * npl : (r + 1) * npl] = results[r]["outT"].T
    return out


LAST_RUNNER = None  # (run_once, fetch) of the most recent kernel() call


def kernel(x, edge_index, edge_weight, lin_w, gcn_w, gamma, beta):
    global LAST_RUNNER
    cfg = _Cfg(N_GLOBAL, NCORES)
    x = np.asarray(x)
    assert x.shape == (cfg.n_global, D)
    K, idx_all, mt_all = _prep_graph(cfg, np.asarray(edge_index), np.asarray(edge_weight))
    in_maps = _prep_inputs(
        cfg, K, idx_all, mt_all, x, np.asarray(lin_w), np.asarray(gcn_w),
        np.asarray(gamma), np.asarray(beta),
    )
    nc = _build_program(cfg, K)
    run_once, fetch = _make_runner(cfg, nc, in_maps)
    LAST_RUNNER = (run_once, fetch)
    results = fetch(run_once())
    return _assemble(cfg, results)
